# revision 8
# baseline (speedup 1.0000x reference)
"""Per-pixel affine transform (bilateral-grid style) on 8 TRN2 NeuronCores.

Reference computation (per batch b, pixel (h, w)):
    out[d] = sum_{c=0..2} x[c] * A[c, d] + A[3, d]
where A[c_in, d] = coeff channel c_in*3 + d.

Sharding: pure data parallel over batch B=8 -> 1 batch per core.
Per-core layout: pixels flattened to [128 partitions, 8192 free]; channels
streamed in groups of 3 (fixed c_in, d=0..2 are contiguous in DRAM).
"""

import os
import sys

for _p in ("/opt/trn_rl_repo",):
    if _p not in sys.path and os.path.isdir(_p):
        sys.path.append(_p)

import numpy as np

import concourse.bacc as bacc
import concourse.mybir as mybir
from concourse.bass_utils import run_bass_kernel_spmd
from concourse.tile import TileContext

B = 8
P = 128          # SBUF partitions
FREE = 8192      # pixels per partition (1024*1024 / 128)
F = 1024         # free-dim chunk
NCHUNK = FREE // F
N_CORES = 8
USE_POOL_ADD = os.environ.get("AFFINE_POOL_ADD", "0") == "1"

_cached_nc = None


def _build_nc():
    nc = bacc.Bacc("TRN2", target_bir_lowering=False, debug=False)
    f32 = mybir.dt.float32

    # (c_in, d, p, f); channel c_in*3+d of the original (12, H, W) coeff
    coeff = nc.dram_tensor("coeff", [4, 3, P, FREE], f32, kind="ExternalInput").ap()
    x = nc.dram_tensor("x", [3, P, FREE], f32, kind="ExternalInput").ap()
    out = nc.dram_tensor("out", [3, P, FREE], f32, kind="ExternalOutput").ap()

    # taper the final chunks so the post-last-load compute tail is short
    widths = [1024] * 7 + [512, 256, 256]
    assert sum(widths) == FREE

    with TileContext(nc) as tc:
        with (
            tc.tile_pool(name="xp", bufs=4) as xp,
            tc.tile_pool(name="ap", bufs=7) as ap_pool,
            tc.tile_pool(name="op", bufs=3) as op,
        ):
            j0 = 0
            for w in widths:
                js = slice(j0, j0 + w)
                j0 += w

                # x chunk: [128, (c f)] with c-major free dim
                X = xp.tile([P, 3 * F], f32)
                nc.sync.dma_start(
                    out=X[:, : 3 * w].rearrange("p (c f) -> p c f", c=3),
                    in_=x[:, :, js].transpose([1, 0, 2]),
                )

                # accumulator starts as the bias group A[3, d]
                OUT = op.tile([P, 3 * F], f32)
                nc.sync.dma_start(
                    out=OUT[:, : 3 * w].rearrange("p (d f) -> p d f", d=3),
                    in_=coeff[3, :, :, js].transpose([1, 0, 2]),
                )

                A_tiles = []
                for c in range(3):
                    A = ap_pool.tile([P, 3 * F], f32)
                    A_tiles.append(A)
                    nc.sync.dma_start(
                        out=A[:, : 3 * w].rearrange("p (d f) -> p d f", d=3),
                        in_=coeff[c, :, :, js].transpose([1, 0, 2]),
                    )
                    xv = X[:, None, c * w : (c + 1) * w].broadcast_to([P, 3, w])
                    Av = A[:, : 3 * w].rearrange("p (d f) -> p d f", d=3)
                    nc.vector.tensor_tensor(Av, Av, xv, mybir.AluOpType.mult)

                # accumulate + store one output channel at a time so stores
                # start while later channels are still summing
                for d in range(3):
                    Od = OUT[:, d * w : (d + 1) * w]
                    sl = slice(d * w, (d + 1) * w)
                    if USE_POOL_ADD:
                        nc.vector.tensor_add(
                            A_tiles[0][:, sl], A_tiles[0][:, sl], A_tiles[1][:, sl]
                        )
                        nc.gpsimd.tensor_add(Od, Od, A_tiles[2][:, sl])
                        nc.vector.tensor_add(Od, Od, A_tiles[0][:, sl])
                    else:
                        for c in range(3):
                            nc.vector.tensor_add(Od, Od, A_tiles[c][:, sl])
                    nc.scalar.dma_start(out=out[d, :, js], in_=Od)
    nc.compile()
    return nc


def _get_nc():
    global _cached_nc
    if _cached_nc is None:
        _cached_nc = _build_nc()
    return _cached_nc


def kernel(coeff, full_res_input):
    coeff = np.ascontiguousarray(coeff, dtype=np.float32)
    x = np.ascontiguousarray(full_res_input, dtype=np.float32)
    assert coeff.shape == (B, 12, 1024, 1024) and x.shape == (B, 3, 1024, 1024)

    nc = _get_nc()
    in_maps = [
        {
            "coeff": coeff[i].reshape(4, 3, P, FREE),
            "x": x[i].reshape(3, P, FREE),
        }
        for i in range(B)
    ]
    res = run_bass_kernel_spmd(nc, in_maps, list(range(N_CORES))).results
    return np.stack([res[i]["out"].reshape(3, 1024, 1024) for i in range(B)])


# revision 9
# speedup vs baseline: 1.0370x; 1.0370x over previous
"""Per-pixel affine transform (bilateral-grid style) on 8 TRN2 NeuronCores.

Reference computation (per batch b, pixel (h, w)):
    out[d] = sum_{c=0..2} x[c] * A[c, d] + A[3, d]
where A[c_in, d] = coeff channel c_in*3 + d.

Sharding: pure data parallel over batch B=8 -> 1 batch per core.
Per-core layout: pixels flattened to [128 partitions, 8192 free]; channels
streamed in groups of 3 (fixed c_in, d=0..2 are contiguous in DRAM).
"""

import os
import sys

for _p in ("/opt/trn_rl_repo",):
    if _p not in sys.path and os.path.isdir(_p):
        sys.path.append(_p)

import numpy as np

import concourse.bacc as bacc
import concourse.mybir as mybir
from concourse.bass_utils import run_bass_kernel_spmd
from concourse.tile import TileContext

B = 8
P = 128          # SBUF partitions
FREE = 8192      # pixels per partition (1024*1024 / 128)
F = 1024         # free-dim chunk
NCHUNK = FREE // F
N_CORES = 8
USE_POOL_ADD = os.environ.get("AFFINE_POOL_ADD", "0") == "1"

_cached_nc = None


def _build_nc():
    nc = bacc.Bacc("TRN2", target_bir_lowering=False, debug=False)
    f32 = mybir.dt.float32

    # (c_in, d, p, f); channel c_in*3+d of the original (12, H, W) coeff
    coeff = nc.dram_tensor("coeff", [4, 3, P, FREE], f32, kind="ExternalInput").ap()
    x = nc.dram_tensor("x", [3, P, FREE], f32, kind="ExternalInput").ap()
    out = nc.dram_tensor("out", [3, P, FREE], f32, kind="ExternalOutput").ap()

    # taper the final chunks so the post-last-load compute tail is short
    widths = [1024] * 7 + [512, 256, 256]
    assert sum(widths) == FREE

    with TileContext(nc) as tc:
        with (
            tc.tile_pool(name="xp", bufs=4) as xp,
            tc.tile_pool(name="ap", bufs=7) as ap_pool,
            tc.tile_pool(name="op", bufs=3) as op,
        ):
            j0 = 0
            for w in widths:
                js = slice(j0, j0 + w)
                j0 += w

                # x chunk: [128, (c f)] with c-major free dim
                X = xp.tile([P, 3 * F], f32)
                nc.sync.dma_start(
                    out=X[:, : 3 * w].rearrange("p (c f) -> p c f", c=3),
                    in_=x[:, :, js].transpose([1, 0, 2]),
                )

                # accumulator starts as the bias group A[3, d]
                OUT = op.tile([P, 3 * F], f32)
                nc.sync.dma_start(
                    out=OUT[:, : 3 * w].rearrange("p (d f) -> p d f", d=3),
                    in_=coeff[3, :, :, js].transpose([1, 0, 2]),
                )

                A_tiles = []
                for c in range(3):
                    A = ap_pool.tile([P, 3 * F], f32)
                    A_tiles.append(A)
                    nc.sync.dma_start(
                        out=A[:, : 3 * w].rearrange("p (d f) -> p d f", d=3),
                        in_=coeff[c, :, :, js].transpose([1, 0, 2]),
                    )
                    xc = X[:, c * w : (c + 1) * w]
                    for d in range(3):
                        Ad = A[:, d * w : (d + 1) * w]
                        nc.vector.tensor_tensor(Ad, Ad, xc, mybir.AluOpType.mult)

                # accumulate + store one output channel at a time so stores
                # start while later channels are still summing
                for d in range(3):
                    Od = OUT[:, d * w : (d + 1) * w]
                    sl = slice(d * w, (d + 1) * w)
                    if USE_POOL_ADD:
                        nc.vector.tensor_add(
                            A_tiles[0][:, sl], A_tiles[0][:, sl], A_tiles[1][:, sl]
                        )
                        nc.gpsimd.tensor_add(Od, Od, A_tiles[2][:, sl])
                        nc.vector.tensor_add(Od, Od, A_tiles[0][:, sl])
                    else:
                        for c in range(3):
                            nc.vector.tensor_add(Od, Od, A_tiles[c][:, sl])
                    nc.scalar.dma_start(out=out[d, :, js], in_=Od)
    nc.compile()
    return nc


def _get_nc():
    global _cached_nc
    if _cached_nc is None:
        _cached_nc = _build_nc()
    return _cached_nc


def kernel(coeff, full_res_input):
    coeff = np.ascontiguousarray(coeff, dtype=np.float32)
    x = np.ascontiguousarray(full_res_input, dtype=np.float32)
    assert coeff.shape == (B, 12, 1024, 1024) and x.shape == (B, 3, 1024, 1024)

    nc = _get_nc()
    in_maps = [
        {
            "coeff": coeff[i].reshape(4, 3, P, FREE),
            "x": x[i].reshape(3, P, FREE),
        }
        for i in range(B)
    ]
    res = run_bass_kernel_spmd(nc, in_maps, list(range(N_CORES))).results
    return np.stack([res[i]["out"].reshape(3, 1024, 1024) for i in range(B)])


# revision 10
# speedup vs baseline: 1.0707x; 1.0325x over previous
"""Per-pixel affine transform (bilateral-grid style) on 8 TRN2 NeuronCores.

Reference computation (per batch b, pixel (h, w)):
    out[d] = sum_{c=0..2} x[c] * A[c, d] + A[3, d]
where A[c_in, d] = coeff channel c_in*3 + d.

Sharding: pure data parallel over batch B=8 -> 1 batch per core.
Per-core layout: pixels flattened to [128 partitions, 8192 free]; channels
streamed in groups of 3 (fixed c_in, d=0..2 are contiguous in DRAM).
"""

import os
import sys

for _p in ("/opt/trn_rl_repo",):
    if _p not in sys.path and os.path.isdir(_p):
        sys.path.append(_p)

import numpy as np

import concourse.bacc as bacc
import concourse.mybir as mybir
from concourse.bass_utils import run_bass_kernel_spmd
from concourse.tile import TileContext

B = 8
P = 128          # SBUF partitions
FREE = 8192      # pixels per partition (1024*1024 / 128)
F = 1024         # free-dim chunk
NCHUNK = FREE // F
N_CORES = 8
USE_POOL_ADD = os.environ.get("AFFINE_POOL_ADD", "0") == "1"

_cached_nc = None


def _build_nc():
    nc = bacc.Bacc("TRN2", target_bir_lowering=False, debug=False)
    f32 = mybir.dt.float32

    # (c_in, d, p, f); channel c_in*3+d of the original (12, H, W) coeff
    coeff = nc.dram_tensor("coeff", [4, 3, P, FREE], f32, kind="ExternalInput").ap()
    x = nc.dram_tensor("x", [3, P, FREE], f32, kind="ExternalInput").ap()
    out = nc.dram_tensor("out", [3, P, FREE], f32, kind="ExternalOutput").ap()

    # taper the final chunks so the post-last-load compute tail is short
    widths = [1024] * 7 + [512, 256, 256]
    assert sum(widths) == FREE

    with TileContext(nc) as tc:
        with (
            tc.tile_pool(name="xp", bufs=4) as xp,
            tc.tile_pool(name="ap", bufs=7) as ap_pool,
            tc.tile_pool(name="op", bufs=3) as op,
        ):
            j0 = 0
            for w in widths:
                js = slice(j0, j0 + w)
                j0 += w

                # x chunk: [128, (c f)] with c-major free dim
                X = xp.tile([P, 3 * F], f32)
                nc.sync.dma_start(
                    out=X[:, : 3 * w].rearrange("p (c f) -> p c f", c=3),
                    in_=x[:, :, js].transpose([1, 0, 2]),
                )

                # accumulator starts as the bias group A[3, d]; issue on the
                # scalar HWDGE ring so descriptor generation overlaps with the
                # sync ring's A-group loads
                OUT = op.tile([P, 3 * F], f32)
                nc.scalar.dma_start(
                    out=OUT[:, : 3 * w].rearrange("p (d f) -> p d f", d=3),
                    in_=coeff[3, :, :, js].transpose([1, 0, 2]),
                )

                A_tiles = []
                for c in range(3):
                    A = ap_pool.tile([P, 3 * F], f32)
                    A_tiles.append(A)
                    nc.sync.dma_start(
                        out=A[:, : 3 * w].rearrange("p (d f) -> p d f", d=3),
                        in_=coeff[c, :, :, js].transpose([1, 0, 2]),
                    )
                    xc = X[:, c * w : (c + 1) * w]
                    for d in range(3):
                        Ad = A[:, d * w : (d + 1) * w]
                        nc.vector.tensor_tensor(Ad, Ad, xc, mybir.AluOpType.mult)

                # accumulate + store one output channel at a time so stores
                # start while later channels are still summing
                for d in range(3):
                    Od = OUT[:, d * w : (d + 1) * w]
                    sl = slice(d * w, (d + 1) * w)
                    if USE_POOL_ADD:
                        nc.vector.tensor_add(
                            A_tiles[0][:, sl], A_tiles[0][:, sl], A_tiles[1][:, sl]
                        )
                        nc.gpsimd.tensor_add(Od, Od, A_tiles[2][:, sl])
                        nc.vector.tensor_add(Od, Od, A_tiles[0][:, sl])
                    else:
                        for c in range(3):
                            nc.vector.tensor_add(Od, Od, A_tiles[c][:, sl])
                    nc.scalar.dma_start(out=out[d, :, js], in_=Od)
    nc.compile()
    return nc


def _get_nc():
    global _cached_nc
    if _cached_nc is None:
        _cached_nc = _build_nc()
    return _cached_nc


def kernel(coeff, full_res_input):
    coeff = np.ascontiguousarray(coeff, dtype=np.float32)
    x = np.ascontiguousarray(full_res_input, dtype=np.float32)
    assert coeff.shape == (B, 12, 1024, 1024) and x.shape == (B, 3, 1024, 1024)

    nc = _get_nc()
    in_maps = [
        {
            "coeff": coeff[i].reshape(4, 3, P, FREE),
            "x": x[i].reshape(3, P, FREE),
        }
        for i in range(B)
    ]
    res = run_bass_kernel_spmd(nc, in_maps, list(range(N_CORES))).results
    return np.stack([res[i]["out"].reshape(3, 1024, 1024) for i in range(B)])
